# revision 10
# baseline (speedup 1.0000x reference)
"""Trainium2 Bass kernel for batched 2D nearest-neighbor retrieval.

Strategy (per core; 8 cores = batch x query-half):
  1. Approximate scores s = 2*q.r - |q|^2 - |r|^2 = -d2 via a K=18 bf16
     triple-split matmul (1 cyc/row on PE; |s - (-d2_fp32)| <= ~1.5e-5,
     measured worst-case true-argmin rank 2 over the whole dataset).
  2. Max-combine tree (DVE tensor_tensor + Pool tensor_tensor) folds the
     16 psum tiles [128,512] down to C3 [128,1024]; cell (u,j) covers the
     8 reals {u*512+j + 1024*c, c=0..7}.
  3. max/max_index give the top-8 cells per query; an indirect DMA
     gathers each cell's 8 candidate rows (rx,ry,rn) from DRAM.
  4. Deferred exact re-score of the 64 candidates with the reference's
     bitwise fp32 arithmetic d2 = (pn+rn) - (2px*rx + 2py*ry), then a
     value/index tournament picks the true argmin.
Host gathers the expression rows by index.
"""
import numpy as np
import concourse.bass as bass
import concourse.tile as tile
from concourse import bacc, mybir
from concourse.bass_utils import run_bass_kernel_spmd

try:
    import ml_dtypes
    BF16 = ml_dtypes.bfloat16
except ImportError:  # pragma: no cover
    import jax.numpy as jnp
    BF16 = jnp.bfloat16

f32 = mybir.dt.float32
bf16 = mybir.dt.bfloat16
u32 = mybir.dt.uint32

B, N, P, G = 4, 8192, 2, 512
QC = N // 2                # queries per core (4096)
NBLK = QC // 128           # 32 query blocks
NT = N // 512              # 16 real tiles
NKEY = 8                   # top-8 cells per query
NC_CAND = 8                # candidates per cell
NCAND = NKEY * NC_CAND     # 64 candidates per query
MAXOP = mybir.AluOpType.max

_cached = {}


def _build():
    nc = bacc.Bacc("TRN2", target_bir_lowering=False, debug=False)

    lhsT_d = nc.dram_tensor("lhsT", [18, QC], bf16, kind="ExternalInput").ap()
    rhs_d = nc.dram_tensor("rhs", [18, N], bf16, kind="ExternalInput").ap()
    cellrows_d = nc.dram_tensor("cellrows", [1024, 4 * NC_CAND], f32,
                                kind="ExternalInput").ap()
    px2e_d = nc.dram_tensor("px2e", [128, NBLK * NCAND], f32, kind="ExternalInput").ap()
    py2e_d = nc.dram_tensor("py2e", [128, NBLK * NCAND], f32, kind="ExternalInput").ap()
    pne_d = nc.dram_tensor("pne", [128, NBLK * NCAND], f32, kind="ExternalInput").ap()
    idx_d = nc.dram_tensor("idx", [128, NBLK], u32, kind="ExternalOutput").ap()

    W = NBLK * NCAND  # 2048, deferred-phase width

    with tile.TileContext(nc) as tc:
        with (
            tc.tile_pool(name="const", bufs=1) as cpool,
            tc.tile_pool(name="cbuf", bufs=2) as cbpool,
            tc.tile_pool(name="small", bufs=3) as spool,
            tc.tile_pool(name="psum", bufs=2, space="PSUM") as ppool,
        ):
            lhsT_sb = cpool.tile([18, QC], bf16, tag="lhsT")
            nc.sync.dma_start(lhsT_sb[:], lhsT_d[:])
            rhs_sb = cpool.tile([18, N], bf16, tag="rhs")
            nc.sync.dma_start(rhs_sb[:], rhs_d[:])
            px2e_sb = cpool.tile([128, W], f32, tag="px2e")
            nc.sync.dma_start(px2e_sb[:], px2e_d[:])
            py2e_sb = cpool.tile([128, W], f32, tag="py2e")
            nc.sync.dma_start(py2e_sb[:], py2e_d[:])
            pne_sb = cpool.tile([128, W], f32, tag="pne")
            nc.sync.dma_start(pne_sb[:], pne_d[:])

            posst = cpool.tile([128, NBLK * NKEY], u32, tag="posst")
            gath = cpool.tile([128, NBLK * NKEY, 4 * NC_CAND], f32, tag="gath")

            for i in range(NBLK):
                qs = bass.ts(i, 128)
                C = cbpool.tile([128, 4096], f32, tag="C")
                for g in range(4):
                    psq = ppool.tile([128, 2048], f32, tag="psq")
                    for t in range(4):
                        nc.tensor.matmul(
                            psq[:, bass.ts(t, 512)], lhsT_sb[:, qs],
                            rhs_sb[:, bass.ts(4 * g + t, 512)],
                            start=True, stop=True)
                    # an op may read only ONE input from PSUM (and Pool
                    # none): Act stages the low half in SBUF, DVE combines
                    acp = cbpool.tile([128, 1024], f32, tag="acp")
                    nc.scalar.activation(
                        acp[:], psq[:, 0:1024],
                        mybir.ActivationFunctionType.Copy)
                    nc.vector.tensor_tensor(
                        C[:, bass.ts(g, 1024)], psq[:, 1024:2048],
                        acp[:], op=MAXOP)
                C2 = cbpool.tile([128, 2048], f32, tag="C2")
                nc.vector.tensor_tensor(
                    C2[:], C[:, 0:2048], C[:, 2048:4096], op=MAXOP)
                C3 = cbpool.tile([128, 1024], f32, tag="C3")
                nc.vector.tensor_tensor(
                    C3[:], C2[:, 0:1024], C2[:, 1024:2048], op=MAXOP)

                v8 = spool.tile([128, 8], f32, tag="v8")
                nc.vector.max(v8[:], C3[:])
                nc.vector.max_index(posst[:, bass.ts(i, 8)], v8[:], C3[:])

                for k in range(NKEY):
                    nc.gpsimd.indirect_dma_start(
                        out=gath[:, i * NKEY + k:i * NKEY + k + 1, :],
                        out_offset=None,
                        in_=cellrows_d[:],
                        in_offset=bass.IndirectOffsetOnAxis(
                            ap=posst[:, i * NKEY + k:i * NKEY + k + 1], axis=0),
                    )

            # ---- deferred exact re-score + tournament ----
            # gath layout per partition: [NBLK*NKEY, NC_CAND, 4] = (rx, ry, rn, 0)
            g4 = gath[:].rearrange("p k (c f) -> p k c f", f=4)
            rx_v = g4[:, :, :, 0:1].squeeze(3)
            ry_v = g4[:, :, :, 1:2].squeeze(3)
            rn_v = g4[:, :, :, 2:3].squeeze(3)

            def as3(ap):  # [128, W] -> [128, NBLK*NKEY, NC_CAND]
                return ap.rearrange("p (k c) -> p k c", c=NC_CAND)

            s1 = cpool.tile([128, NBLK * NKEY, NC_CAND], f32, tag="s1")
            s2 = cpool.tile([128, NBLK * NKEY, NC_CAND], f32, tag="s2")
            # c2 = fl(fl(2px*rx) + fl(2py*ry)); d2 = fl(fl(pn+rn) - c2)
            nc.vector.tensor_tensor(s1[:], rx_v, as3(px2e_sb[:]), op=mybir.AluOpType.mult)
            nc.vector.tensor_tensor(s2[:], ry_v, as3(py2e_sb[:]), op=mybir.AluOpType.mult)
            nc.vector.tensor_tensor(s1[:], s1[:], s2[:], op=mybir.AluOpType.add)
            nc.vector.tensor_tensor(s2[:], rn_v, as3(pne_sb[:]), op=mybir.AluOpType.add)
            nc.vector.tensor_tensor(s1[:], s2[:], s1[:], op=mybir.AluOpType.subtract)
            # s1 = exact d2 of all 64 candidates per query

            # candidate real-index (as f32): ridx = pos + 1024*c
            posf = cpool.tile([128, NBLK * NKEY], f32, tag="posf")
            nc.vector.tensor_copy(posf[:], posst[:])
            ridx = cpool.tile([128, NBLK * NKEY, NC_CAND], f32, tag="ridx")
            for c in range(NC_CAND):
                nc.vector.tensor_scalar(
                    ridx[:, :, c:c + 1].squeeze(2), posf[:],
                    float(1024 * c), None, op0=mybir.AluOpType.add)

            # tournament min over the 64 candidates of each block
            def level(vin, iin, n, tag):
                # vin/iin: AP views [128, NBLK, n]; returns views with n//2
                vo = cpool.tile([128, NBLK, n // 2], f32, tag=f"v{tag}")
                io = cpool.tile([128, NBLK, n // 2], f32, tag=f"i{tag}")
                m = cpool.tile([128, NBLK, n // 2], mybir.dt.uint8, tag=f"m{tag}")
                vp = vin.rearrange("p b (n two) -> p b n two", two=2)
                ip = iin.rearrange("p b (n two) -> p b n two", two=2)
                vL = vp[:, :, :, 0:1].squeeze(3)
                vR = vp[:, :, :, 1:2].squeeze(3)
                iL = ip[:, :, :, 0:1].squeeze(3)
                iR = ip[:, :, :, 1:2].squeeze(3)
                nc.vector.tensor_tensor(m[:], vL, vR, op=mybir.AluOpType.is_le)
                nc.vector.tensor_copy(vo[:], vR)
                nc.vector.copy_predicated(vo[:], m[:], vL)
                nc.vector.tensor_copy(io[:], iR)
                nc.vector.copy_predicated(io[:], m[:], iL)
                return vo[:], io[:]

            vv = s1[:].rearrange("p k c -> p (k c)").rearrange(
                "p (b n) -> p b n", n=NCAND)
            iv = ridx[:].rearrange("p k c -> p (k c)").rearrange(
                "p (b n) -> p b n", n=NCAND)
            n = NCAND
            li = 0
            while n > 1:
                vv, iv = level(vv, iv, n, li)
                n //= 2
                li += 1

            idxu = spool.tile([128, NBLK], u32, tag="idxu")
            nc.vector.tensor_copy(idxu[:], iv.rearrange("p b n -> p (b n)"))
            nc.sync.dma_start(idx_d[:], idxu[:])

    nc.compile()
    return nc


def _split3(x):
    x = np.asarray(x, np.float32)
    h = x.astype(BF16).astype(np.float32)
    r = (x - h).astype(np.float32)
    m = r.astype(BF16).astype(np.float32)
    l = (r - m).astype(np.float32).astype(BF16).astype(np.float32)
    return h, m, l


def _prep_core(pred_bh, real_b):
    """pred_bh [QC,2] f32, real_b [N,2] f32 -> input map for one core."""
    px = pred_bh[:, 0].astype(np.float32)
    py = pred_bh[:, 1].astype(np.float32)
    rx = real_b[:, 0].astype(np.float32)
    ry = real_b[:, 1].astype(np.float32)
    pnn = (px * px + py * py).astype(np.float32)
    rnn = (rx * rx + ry * ry).astype(np.float32)

    ah, am, al = _split3(2.0 * px)
    bh, bm, bl = _split3(2.0 * py)
    ph, pm, pl = _split3(-pnn)
    qh, qm, ql = _split3(-rnn)
    ch, cm, cl = _split3(rx)
    dh, dm, dl = _split3(ry)
    onesQ = np.ones(QC, np.float32)
    onesN = np.ones(N, np.float32)
    zQ = np.zeros(QC, np.float32)
    zN = np.zeros(N, np.float32)

    # row k contributes lhsT[k] (per query) * rhs[k] (per real)
    lrows = [ph, onesQ, ah, bh, pm, onesQ, ah, am, bh, bm, pl, onesQ,
             ah, al, am, bh, bl, bm]
    rrows = [onesN, qh, ch, dh, onesN, qm, cm, ch, dm, dh, onesN, ql,
             cl, ch, cm, dl, dh, dm]
    lhsT = np.stack(lrows).astype(BF16)
    rhs = np.stack(rrows).astype(BF16)

    # gather table: row pos = (u*512+j) -> 8 candidates (u+2c)*512+j
    r_all = np.stack([rx, ry, rnn, zN], axis=-1)          # [N, 4]
    cr = r_all.reshape(16, 512, 4)                        # [tile, j, 4]
    rows = np.empty((2, 512, NC_CAND, 4), np.float32)
    for u in range(2):
        rows[u] = cr[u::2].transpose(1, 0, 2)             # [512, 8, 4]
    cellrows = np.ascontiguousarray(rows.reshape(1024, 4 * NC_CAND))

    # expanded per-(partition, blk, cand) query data; query q = blk*128 + p
    def expand(v):
        # v [QC] -> [128, NBLK*NCAND]
        m = v.reshape(NBLK, 128).T                        # [128, NBLK]
        return np.ascontiguousarray(
            np.repeat(m[:, :, None], NCAND, axis=2).reshape(128, NBLK * NCAND))

    return {
        "lhsT": np.ascontiguousarray(lhsT),
        "rhs": np.ascontiguousarray(rhs),
        "cellrows": cellrows,
        "px2e": expand((2.0 * px).astype(np.float32)),
        "py2e": expand((2.0 * py).astype(np.float32)),
        "pne": expand(pnn),
    }


def kernel(predicted_positions, real_positions, real_expressions):
    pred = np.ascontiguousarray(predicted_positions, dtype=np.float32)
    real = np.ascontiguousarray(real_positions, dtype=np.float32)
    expr = np.asarray(real_expressions)

    if "nc" not in _cached:
        _cached["nc"] = _build()
    nc = _cached["nc"]

    in_maps = []
    for c in range(8):
        b, h = c // 2, c % 2
        in_maps.append(_prep_core(pred[b, h * QC:(h + 1) * QC], real[b]))

    results = run_bass_kernel_spmd(nc, in_maps, list(range(8))).results

    out = np.empty((B, N, G), dtype=expr.dtype)
    for c in range(8):
        b, h = c // 2, c % 2
        idx = results[c]["idx"].T.reshape(QC).astype(np.int64)  # [QC]
        out[b, h * QC:(h + 1) * QC] = expr[b, idx]
    return out
